# revision 13
# baseline (speedup 1.0000x reference)
"""Trainium2 Bass kernel for nn_Attention_27358941675773.

Reference computation (per batch b):
    q = x @ Q              [N, H]
    k = x @ K              [N, H]
    V = V_down @ V_up      [L, L]
    v = x @ V              [N, L]
    S = q @ k.T / 256      [N, N]
    out = softmax(S) @ v   [N, L]

Sharding: pure data-parallel over batch B=8 across the 8 NeuronCores
(one batch element per core); small params replicated. No collectives.

Per-core kernel design (N=4096, L=256, H=128):
  - Inputs shipped as fp16 (x transposed to [L, N]); all matmuls run at
    full PE rate. qT [H,N] / kT [H,N] are computed directly transposed so
    scores are built as S_T[m, n] (keys on partitions), no transposes.
  - Value path factored through the rank-H bottleneck:
        out = softmax(S) @ x @ V_down @ V_up
    so the O(N^2) product contracts into H=128 columns.
  - PSUM layout (8 banks): one 3-slot score ring [128, 3, 1024] f32
    (6 banks, manually indexed) + mid accumulator [128, 1024] f32
    (2 banks). exp runs on PAIRS of score tiles via a 3-dim AP over the
    ring (strides +4KB or -8KB), halving the per-instruction overhead on
    the Scalar engine, which paces the whole kernel.
  - Rowsum of exp-scores: a pairwise tree of 2048-wide bf16 adds on the
    Vector engine over the 15 leading pairs (17 ops/block instead of 31
    narrow ones), with the last pair folded separately so the
    post-last-exp chain is only two 1024-wide adds.
  - Partition-axis reduction+broadcast of the rowsum in ONE matmul with
    an all-ones [128,128] stationary operand (fp32r, full PE rate),
    replacing the 8.3us GpSimd PartitionAllReduce.
  - Normalization applied after V_up at the output-copy stage; the
    numerator copy (mid -> SBUF) rides the Vector engine, keeping the
    Scalar engine exclusively on the exp stream.
  - DMA: x arrives in 4 large descriptors on the SP queue; weights in 4
    single-issue strided descriptors on the GpSimd queue - the serialized
    ~0.7us-per-issue descriptor cost no longer delays the first exp.
  - Uniform half-block-lagged schedule as before: per pair-slot the PE
    runs 4 QK matmuls + 4 lagged attention@w matmuls; block 0 uses the
    projections as filler; ~10 junk matmuls warm the PE clock gate.
  - Output stored transposed [L, N] fp16; host un-transposes on gather.
"""

import os
import sys

import numpy as np

for _p in ("/opt/trn_rl_repo",):
    if _p not in sys.path and os.path.isdir(_p):
        sys.path.insert(0, _p)

B, N, L, H = 8, 4096, 256, 128
SCALER = 256.0
NB = 1024           # query-block (free dim of score tiles)
NBH = 512           # half tile (one PSUM bank of fp32)
NT = N // NB        # 4 query blocks
MT = N // 128       # 32 key tiles of 128
NP = 16             # key-tile PAIRS per block
P = 128


def _build():
    import concourse.bass as bass
    import concourse.tile as tile
    from concourse import bacc, bass_isa, mybir
    from contextlib import ExitStack

    f32 = mybir.dt.float32
    f32r = mybir.dt.float32r
    f16 = mybir.dt.float16
    bf16 = mybir.dt.bfloat16
    AF = mybir.ActivationFunctionType

    nc = bacc.Bacc(
        "TRN2", target_bir_lowering=False, debug=False, num_devices=B
    )

    xT_ext = nc.declare_dram_parameter("xT", [L, N], f16, isOutput=False)
    wq_ext = nc.declare_dram_parameter("Wq", [L, H], f16, isOutput=False)
    wk_ext = nc.declare_dram_parameter("Wk", [L, H], f16, isOutput=False)
    vd_ext = nc.declare_dram_parameter("Vd", [L, H], f16, isOutput=False)
    vu_ext = nc.declare_dram_parameter("Vu", [H, L], f16, isOutput=False)
    # output stored transposed [L, N]; host un-transposes at gather
    out_ext = nc.declare_dram_parameter("out", [L, N], f16, isOutput=True)

    with tile.TileContext(nc) as tc, ExitStack() as ctx:
        persist = ctx.enter_context(tc.tile_pool(name="persist", bufs=1))

        ones32f = persist.tile([P, P], f32)
        nc.gpsimd.memset(ones32f[:], 1.0)
        ones32 = persist.tile([P, P], f32r)
        nc.vector.tensor_copy(ones32[:], ones32f[:])
        # touch Exp right away so the ~2.7us ACT table load overlaps the
        # input DMAs instead of delaying the first real exp
        dum = persist.tile([1, 2], f32)
        nc.gpsimd.memset(dum[:], 0.0)
        nc.scalar.activation(dum[:, 1:2], dum[:, 0:1], AF.Exp)
        wrm = persist.tile([P, NBH], bf16, name="wrm")
        nc.vector.memset(wrm[:], 0.0)

        qw16 = persist.tile([P, 2 * H], f16)    # Q   [l_chunk][l_in, h]
        kw16 = persist.tile([P, 2 * H], f16)
        vd16 = persist.tile([P, 2 * H], f16)    # V_down [l_chunk][l_in, h]
        vu16 = persist.tile([P, L], f16)        # V_up   [h, l]
        vu_bf = persist.tile([P, L], bf16)      # V_up as bf16 (out matmul)
        xt16 = [persist.tile([P, N], f16, name=f"xt16_{c}") for c in range(2)]
        qT16 = persist.tile([P, N], f16)        # q.T       [h, n]
        kT16 = persist.tile([P, N], f16)        # k.T       [h, m]
        w_sb = persist.tile([P, MT * H], bf16)  # x@V_down  [m_tile][m_in, h]

        # ---------------- phase A: direct fp16 loads ----------------
        # x s0 chunks first (critical path for the first QK tiles), as
        # two large descriptors; the rest of x as two more. Weights ride
        # the GpSimd queue concurrently, one descriptor per tensor.
        for w_ext, w_sbuf in ((wq_ext, qw16), (wk_ext, kw16), (vd_ext, vd16)):
            nc.gpsimd.dma_start(
                w_sbuf[:].rearrange("p (c h) -> p c h", c=2),
                w_ext[:, :].rearrange("(c p) h -> p c h", c=2),
            )
        for c in range(2):
            nc.sync.dma_start(
                xt16[c][:, 0:NB], xT_ext[c * P:(c + 1) * P, 0:NB]
            )
        nc.gpsimd.dma_start(vu16[:], vu_ext[:, :])
        for c in range(2):
            nc.sync.dma_start(
                xt16[c][:, NB:N], xT_ext[c * P:(c + 1) * P, NB:N]
            )

        # ------------- phases B+C: projections fused with attention -------
        with (
            tc.tile_pool(name="psum", bufs=1, space="PSUM") as psum_pool,
            tc.tile_pool(name="est", bufs=8) as est_pool,
            tc.tile_pool(name="tree", bufs=2) as tree_pool,
            tc.tile_pool(name="sb_small", bufs=2) as sb_small,
            tc.tile_pool(name="outfin", bufs=2) as outfin_pool,
        ):
            # manual PSUM layout: 3-slot score ring (6 banks) + aux (2
            # banks, time-shared between the mid accumulator and the
            # block-0 w/qkT projection staging)
            sr = psum_pool.tile([P, 3, NB], f32, name="score_ring")
            aux = psum_pool.tile([P, NB], f32, name="aux")
            gslot = [0]

            def slot():
                i = gslot[0] % 3
                gslot[0] += 1
                return i

            estP = {}     # (k, g) -> bf16 [128, 2048] pair exp tiles
            estS = {}     # (k, g) -> bf16 [128, 1024] single exp tiles
            mscs = {}     # k -> normalized mid (bf16, SBUF)
            bc = {}       # k -> [128, NB] f32 broadcast 1/rowsum (SBUF)
            tr = {}       # tree tiles by (k, name)

            def est_ap(k, t, h):
                # 512-wide slice of the exp tile holding key-tile t
                g, r = divmod(t, 3) if t < 30 else (10, t - 30)
                if g < 10 and r == 2:
                    return estS[(k, g)][:, h * NBH:(h + 1) * NBH]
                off = r * NB + h * NBH
                return estP[(k, g)][:, off:off + NBH]

            def proj_qkT_pair(w16, dst, f, where, on_act=False):
                # projects halves f and f+1; one copy. where: ring|aux
                ps = sr[:, slot(), :] if where == "ring" else aux[:, :]
                for half in range(2):
                    ff = f + half
                    for c in range(2):
                        nc.tensor.matmul(
                            ps[half * NBH:(half + 1) * NBH]
                            if False else ps[:, half * NBH:(half + 1) * NBH],
                            w16[:, c * H:(c + 1) * H],
                            xt16[c][:, ff * NBH:(ff + 1) * NBH],
                            start=(c == 0), stop=(c == 1),
                        )
                if on_act:
                    nc.scalar.activation(
                        dst[:, f * NBH:(f + 2) * NBH], ps, AF.Copy
                    )
                else:
                    nc.vector.tensor_copy(dst[:, f * NBH:(f + 2) * NBH], ps)

            def proj_w_batch8(b):
                # w tiles 8b..8b+7 staged in one ring slot, one copy
                ps = sr[:, slot(), :]
                for j8 in range(8):
                    j = 8 * b + j8
                    for c in range(2):
                        nc.tensor.matmul(
                            ps[:, j8 * H:(j8 + 1) * H],
                            xt16[c][:, j * P:(j + 1) * P],
                            vd16[:, c * H:(c + 1) * H],
                            start=(c == 0), stop=(c == 1),
                        )
                nc.vector.tensor_copy(
                    w_sb[:, b * NB:(b + 1) * NB], ps
                )

            def qk_exp_P(k, g):
                t0 = 3 * g if g < 10 else 30
                # keep P-pairs contiguous -- a (2,0) wrap pair has a
                # bounding AP range spanning all three slots, which the
                # overlap tracker treats as a full-ring dependency
                if gslot[0] % 3 == 2:
                    gslot[0] += 1
                i0 = slot()
                i1 = slot()
                for t, i in ((t0, i0), (t0 + 1, i1)):
                    for h in range(2):
                        nc.tensor.matmul(
                            sr[:, i, h * NBH:(h + 1) * NBH],
                            kT16[:, t * P:(t + 1) * P],
                            qT16[:, k * NB + h * NBH: k * NB + (h + 1) * NBH],
                            start=True, stop=True,
                        )
                src_ap = sr[:, i0:i0 + 2, :]
                e = est_pool.tile([P, 2 * NB], bf16, tag="estP", bufs=8,
                                  name=f"estP_{k}_{g}")
                estP[(k, g)] = e
                nc.scalar.activation(e[:], src_ap, AF.Exp, scale=1.0 / SCALER)

            def qk_exp_S(k, g):
                t = 3 * g + 2
                i = slot()
                for h in range(2):
                    nc.tensor.matmul(
                        sr[:, i, h * NBH:(h + 1) * NBH],
                        kT16[:, t * P:(t + 1) * P],
                        qT16[:, k * NB + h * NBH: k * NB + (h + 1) * NBH],
                        start=True, stop=True,
                    )
                e = est_pool.tile([P, NB], bf16, tag="estS", bufs=8,
                                  name=f"estS_{k}_{g}")
                estS[(k, g)] = e
                nc.scalar.activation(e[:], sr[:, i, :], AF.Exp,
                                     scale=1.0 / SCALER)

            # ---- PV: FIFO queue of (k, j), popped on a per-group budget
            pvq = []
            pvhead = [0]

            def norm_mid(k):
                msc = sb_small.tile([P, NB], bf16, tag="msc", bufs=2,
                                    name=f"msc_{k}")
                nc.vector.tensor_copy(msc[:], aux[:, :])
                mscs[k] = msc

            def emit_pv(n):
                for _ in range(n):
                    if pvhead[0] >= len(pvq):
                        return
                    kk, j = pvq[pvhead[0]]
                    pvhead[0] += 1
                    for h in range(2):
                        nc.tensor.matmul(
                            aux[:, h * NBH:(h + 1) * NBH],
                            w_sb[:, j * H:(j + 1) * H],
                            est_ap(kk, j, h),
                            start=(j == 0), stop=(j == MT - 1),
                        )
                    if j == MT - 1:
                        norm_mid(kk)

            def tadd(k, name, a, b, dtype, width):
                nb = 1 if name in ("c0", "pp", "sss", "sp", "part") else 2
                t = tree_pool.tile([P, width], dtype, tag=name.rstrip(
                    "0123456789") or name, bufs=nb, name=f"{name}_{k}")
                nc.vector.tensor_add(t[:], a, b)
                tr[(k, name)] = t
                return t

            def tree_adds(k, g):
                # P-chain (2048-wide) + S-chain (1024-wide), bf16
                W2, W1 = 2 * NB, NB
                if g % 2 == 1:
                    i = g // 2
                    tadd(k, f"a{i}", estP[(k, g - 1)][:], estP[(k, g)][:],
                         bf16, W2)
                    tadd(k, f"s{i}", estS[(k, g - 1)][:], estS[(k, g)][:],
                         bf16, W1)
                if g == 3:
                    tadd(k, "b0", tr[(k, "a0")][:], tr[(k, "a1")][:], bf16, W2)
                    tadd(k, "ss0", tr[(k, "s0")][:], tr[(k, "s1")][:], bf16, W1)
                if g == 7:
                    tadd(k, "b1", tr[(k, "a2")][:], tr[(k, "a3")][:], bf16, W2)
                    tadd(k, "ss1", tr[(k, "s2")][:], tr[(k, "s3")][:], bf16, W1)
                    tadd(k, "c0", tr[(k, "b0")][:], tr[(k, "b1")][:], bf16, W2)
                if g == 9:
                    pp = tadd(k, "pp", tr[(k, "c0")][:], tr[(k, "a4")][:],
                              bf16, W2)
                    tadd(k, "sss", tr[(k, "ss0")][:], tr[(k, "ss1")][:],
                         bf16, W1)
                    sp = tadd(k, "sp", tr[(k, "sss")][:], tr[(k, "s4")][:],
                              bf16, W1)
                    pf = tree_pool.tile([P, NB], f32, tag="pf", bufs=1,
                                        name=f"pf_{k}")
                    nc.vector.tensor_add(pf[:], pp[:, 0:NB], pp[:, NB:2 * NB])
                    tr[(k, "pf")] = pf
                if g == 10:
                    tadd(k, "part", tr[(k, "pf")][:], tr[(k, "sp")][:],
                         f32, W1)

            def fold_last(k):
                # fold the final pair (tiles 30,31) into the rowsum
                p10f = tree_pool.tile([P, NB], f32, tag="p10f", bufs=1,
                                      name=f"p10f_{k}")
                nc.vector.tensor_add(
                    p10f[:], estP[(k, 10)][:, 0:NB], estP[(k, 10)][:, NB:2 * NB]
                )
                t = tree_pool.tile([P, NB], f32r, tag="t5", bufs=1,
                                   name=f"t5_{k}")
                nc.vector.tensor_add(t[:], tr[(k, "part")][:], p10f[:])
                tr[(k, "t5")] = t

            def bc_chain(k):
                # partition-sum + broadcast in one all-ones fp32r matmul
                i = slot()
                for h in range(2):
                    nc.tensor.matmul(
                        sr[:, i, h * NBH:(h + 1) * NBH],
                        ones32[:],
                        tr[(k, "t5")][:, h * NBH:(h + 1) * NBH],
                        start=True, stop=True,
                    )
                bck = sb_small.tile([P, NB], f32, tag="bc", bufs=2,
                                    name=f"bc_{k}")
                nc.vector.reciprocal_approx_fast(bck[:], sr[:, i, :])
                bc[k] = bck

            def drain_out(k):
                for lt in range(2):
                    i = slot()
                    for h in range(2):
                        nc.tensor.matmul(
                            sr[:, i, h * NBH:(h + 1) * NBH],
                            vu_bf[:, lt * P:(lt + 1) * P],
                            mscs[k][:, h * NBH:(h + 1) * NBH],
                            start=True, stop=True,
                        )
                    fin = outfin_pool.tile([P, NB], f16, tag="fin")
                    nc.vector.tensor_mul(fin[:], sr[:, i, :], bc[k][:])
                    nc.gpsimd.dma_start(
                        out_ext[lt * P:(lt + 1) * P, k * NB:(k + 1) * NB],
                        fin[:],
                    )

            # PE warm-up while the x DMA is in flight
            for _ in range(10):
                i = slot()
                nc.tensor.matmul(
                    sr[:, i, 0:NBH], wrm[:, :P], wrm[:], start=True, stop=True
                )

            # head: first QK tiles need qT/kT half-blocks 0,1 (chunk s0)
            proj_qkT_pair(qw16, qT16, 0, "ring", on_act=True)
            proj_qkT_pair(kw16, kT16, 0, "ring", on_act=False)

            # per-group PV budgets: one-group lag. Each block pops its
            # last 2 js at the next block's g0 (msc rides that pop).
            BUD0 = [0, 0, 3, 3, 3, 3, 3, 3, 3, 3, 3]   # 27, leftover 5
            BUD1 = [5, 3, 3, 3, 3, 3, 3, 3, 3, 3, 3]   # catches up
            BUD = [2, 3, 3, 3, 3, 3, 3, 3, 3, 3, 3]

            for k in range(NT):
                pvq.extend((k, j) for j in range(MT))
                bud = BUD0 if k == 0 else (BUD1 if k == 1 else BUD)
                for g in range(11):
                    qk_exp_P(k, g)
                    if g < 10:
                        qk_exp_S(k, g)
                    emit_pv(bud[g])
                    if k == 0:
                        # projection fillers (w batches + later qkT halves)
                        if g in (0, 1):
                            proj_w_batch8(g)
                        if g == 0:
                            proj_qkT_pair(kw16, kT16, 2, "aux")
                        if g == 1:
                            proj_qkT_pair(kw16, kT16, 4, "aux")
                        if g == 2:
                            proj_qkT_pair(kw16, kT16, 6, "ring")
                        if g == 3:
                            proj_w_batch8(2)
                        if g == 4:
                            proj_qkT_pair(qw16, qT16, 2, "ring")
                        if g == 5:
                            proj_w_batch8(3)
                            nc.vector.tensor_copy(vu_bf[:], vu16[:])
                        if g == 6:
                            proj_qkT_pair(qw16, qT16, 4, "ring")
                        if g == 8:
                            proj_qkT_pair(qw16, qT16, 6, "ring")
                    if k >= 1:
                        if g == 0:
                            fold_last(k - 1)
                        if g == 1:
                            bc_chain(k - 1)
                        if g == 2:
                            drain_out(k - 1)
                    tree_adds(k, g)

            # epilogue: drain the PV queue, block-3 rowsum chain, output
            k3 = NT - 1
            emit_pv(len(pvq) - pvhead[0])
            fold_last(k3)
            bc_chain(k3)
            drain_out(k3)

    if not nc.is_finalized():
        nc.finalize()
    return nc


_GRAPH_CACHE = {}


def _get_graph():
    if "nc" not in _GRAPH_CACHE:
        _GRAPH_CACHE["nc"] = _build()
    return _GRAPH_CACHE["nc"]


def run(inputs: dict, trace: bool = False):
    """Run the SPMD kernel on 8 cores. Returns (output, BassKernelResults)."""
    from concourse.bass_utils import run_bass_kernel_spmd

    x = np.asarray(inputs["x"], dtype=np.float32)
    Q = np.asarray(inputs["Q"], dtype=np.float32)[0]
    K = np.asarray(inputs["K"], dtype=np.float32)[0]
    Vd = np.asarray(inputs["V_down"], dtype=np.float32)[0]
    Vu = np.asarray(inputs["V_up"], dtype=np.float32)[0]

    wq = np.ascontiguousarray(Q).astype(np.float16)
    wk = np.ascontiguousarray(K).astype(np.float16)
    vd = np.ascontiguousarray(Vd).astype(np.float16)
    vu = np.ascontiguousarray(Vu).astype(np.float16)

    in_maps = []
    for b in range(B):
        in_maps.append({
            "xT": np.ascontiguousarray(x[b].T).astype(np.float16),
            "Wq": wq,
            "Wk": wk,
            "Vd": vd,
            "Vu": vu,
        })

    nc = _get_graph()
    res = run_bass_kernel_spmd(nc, in_maps, core_ids=list(range(B)), trace=trace)
    # device output is [L, N] per core; un-transpose during the gather
    out = np.stack([np.asarray(res.results[i]["out"]).astype(np.float32).T for i in range(B)])
    return np.ascontiguousarray(out, dtype=np.float32), res


def kernel(**inputs) -> np.ndarray:
    out, _ = run(inputs, trace=False)
    return out


# revision 15
# speedup vs baseline: 1.7905x; 1.7905x over previous
"""Trainium2 Bass kernel for nn_Attention_27358941675773.

Reference computation (per batch b):
    q = x @ Q              [N, H]
    k = x @ K              [N, H]
    V = V_down @ V_up      [L, L]
    v = x @ V              [N, L]
    S = q @ k.T / 256      [N, N]
    out = softmax(S) @ v   [N, L]

Sharding: pure data-parallel over batch B=8 across the 8 NeuronCores
(one batch element per core); small params replicated. No collectives.

Per-core kernel design (N=4096, L=256, H=128):
  - Inputs shipped as fp16 (x transposed to [L, N]); all matmuls run at
    full PE rate. qT [H,N] and kT [H,N] are computed directly in
    transposed layout so scores are built as S_T[m, n] (keys on the
    partition axis) with no transposes anywhere in the pipeline.
  - Value path factored through the rank-H bottleneck:
        out = softmax(S) @ x @ V_down @ V_up
    so the O(N^2) product contracts into H=128 columns and V_up is
    applied after the softmax.
  - exp(S_T/256) runs on the Scalar engine straight out of PSUM in
    [128, 1024] tiles through a 3-slot PSUM ring (the scheduler's WAR
    semaphores release one exp late, so a ring depth of 3 is the
    minimum that keeps QK fully pipelined against the exp stream; wider
    exp tiles are geometrically impossible with 8 PSUM banks and the
    2-bank PV accumulator). The exp stream is the critical path.
  - exp output lands in PAIRED [128, 2048] est tiles (two exps fill one
    tile), letting the rowsum tree run 2048-wide bf16 adds on the
    Vector engine: 16 tree ops per block instead of 31, and the last
    pair folds separately so the post-last-exp chain is short.
  - Partition-axis rowsum reduce+broadcast in ONE matmul with an
    all-ones [128,128] fp32r stationary operand (full PE rate),
    replacing the 8.3us GpSimd PartitionAllReduce.
  - The numerator copy (mid -> SBUF) and normalization ride the Vector
    engine; the Scalar engine runs nothing but the exp stream.
  - DMA: weights first as single strided descriptors on the GpSimd
    queue, then x in 4 large descriptors on the SP queue, so the
    projection weights never queue behind the 2MB x transfer.
  - Uniform half-block-lagged schedule: per key tile the PE runs 2 QK
    matmuls plus 2 lagged attention@w matmuls; block 0 uses the
    projections (batched 4-8 to a PSUM slot) as its filler; junk
    matmuls warm the PE clock gate during the input DMA.
  - Output stored transposed [L, N] fp16; host un-transposes on gather.
"""

import os
import sys

import numpy as np

for _p in ("/opt/trn_rl_repo",):
    if _p not in sys.path and os.path.isdir(_p):
        sys.path.insert(0, _p)

B, N, L, H = 8, 4096, 256, 128
SCALER = 256.0
NB = 1024           # query-block (free dim of score tiles)
NBH = 512           # half block (one PSUM bank of fp32)
NT = N // NB        # 4 query blocks
MT = N // 128       # 32 key tiles of 128
P = 128


def _build():
    import concourse.bass as bass
    import concourse.tile as tile
    from concourse import bacc, bass_isa, mybir
    from contextlib import ExitStack

    f32 = mybir.dt.float32
    f32r = mybir.dt.float32r
    f16 = mybir.dt.float16
    bf16 = mybir.dt.bfloat16
    AF = mybir.ActivationFunctionType

    nc = bacc.Bacc(
        "TRN2", target_bir_lowering=False, debug=False, num_devices=B
    )

    xT_ext = nc.declare_dram_parameter("xT", [L, N], f16, isOutput=False)
    wq_ext = nc.declare_dram_parameter("Wq", [L, H], f16, isOutput=False)
    wk_ext = nc.declare_dram_parameter("Wk", [L, H], f16, isOutput=False)
    vd_ext = nc.declare_dram_parameter("Vd", [L, H], f16, isOutput=False)
    vu_ext = nc.declare_dram_parameter("Vu", [H, L], f16, isOutput=False)
    # output stored transposed [L, N]; host un-transposes at gather
    out_ext = nc.declare_dram_parameter("out", [L, N], f16, isOutput=True)

    with tile.TileContext(nc) as tc, ExitStack() as ctx:
        persist = ctx.enter_context(tc.tile_pool(name="persist", bufs=1))

        ones32f = persist.tile([P, P], f32)
        nc.gpsimd.memset(ones32f[:], 1.0)
        ones32 = persist.tile([P, P], f32r)
        nc.vector.tensor_copy(ones32[:], ones32f[:])
        # touch Exp right away so the ~2.7us ACT table load overlaps the
        # input DMAs instead of delaying the first real exp
        dum = persist.tile([1, 2], f32)
        nc.gpsimd.memset(dum[:], 0.0)
        nc.scalar.activation(dum[:, 1:2], dum[:, 0:1], AF.Exp)
        wrm = persist.tile([P, NBH], bf16, name="wrm")
        nc.vector.memset(wrm[:], 0.0)

        qw16 = persist.tile([P, 2 * H], f16)    # Q   [l_chunk][l_in, h]
        kw16 = persist.tile([P, 2 * H], f16)
        vd16 = persist.tile([P, 2 * H], f16)    # V_down [l_chunk][l_in, h]
        vu16 = persist.tile([P, L], f16)        # V_up   [h, l]
        vu_bf = persist.tile([P, L], bf16)      # V_up as bf16 (out matmul)
        xt16 = [persist.tile([P, N], f16, name=f"xt16_{c}") for c in range(2)]
        qT16 = persist.tile([P, N], f16)        # q.T       [h, n]
        kT16 = persist.tile([P, N], f16)        # k.T       [h, m]
        w_sb = persist.tile([P, MT * H], bf16)  # x@V_down  [m_tile][m_in, h]

        # ---------------- phase A: direct fp16 loads ----------------
        # weights first (single strided descriptors on the GpSimd queue)
        # so they never queue behind the 2MB x transfer; x s0 chunks next
        # (critical path for the first QK tiles), then the rest of x.
        for w_ext, w_sbuf in ((wq_ext, qw16), (wk_ext, kw16), (vd_ext, vd16)):
            nc.gpsimd.dma_start(
                w_sbuf[:].rearrange("p (c h) -> p c h", c=2),
                w_ext[:, :].rearrange("(c p) h -> p c h", c=2),
            )
        for c in range(2):
            nc.sync.dma_start(
                xt16[c][:, 0:NB], xT_ext[c * P:(c + 1) * P, 0:NB]
            )
        nc.gpsimd.dma_start(vu16[:], vu_ext[:, :])
        for c in range(2):
            nc.sync.dma_start(
                xt16[c][:, NB:N], xT_ext[c * P:(c + 1) * P, NB:N]
            )

        # ------------- phases B+C: projections fused with attention -------
        with (
            tc.tile_pool(name="est", bufs=20) as est_pool,
            tc.tile_pool(name="tree", bufs=2) as tree_pool,
            tc.tile_pool(name="sb_small", bufs=2) as sb_small,
            tc.tile_pool(name="outfin", bufs=4) as outfin_pool,
            tc.tile_pool(name="stp", bufs=3, space="PSUM") as stp,
            tc.tile_pool(name="mtp", bufs=1, space="PSUM") as mtp,
        ):
            est = {}      # (k, pair) -> bf16 [128, 2048] exp tiles (2 halves)
            mtiles = {}   # k -> psum numerator mid^T [h, n] tile
            mscs = {}     # k -> normalized mid (bf16, SBUF)
            bc = {}       # k -> [128, NB] f32 broadcast 1/rowsum
            tr = {}       # tree tiles by (k, name)

            def est_ap(k, j, h):
                off = (j % 2) * NB + h * NBH
                return est[(k, j // 2)][:, off:off + NBH]

            def proj_qkT_pair(w16, dst, f):
                # projects halves f and f+1 into one psum slot, one copy
                ps = stp.tile([P, NB], f32, tag="stp", name=f"pjp_{f}")
                for half in range(2):
                    ff = f + half
                    for c in range(2):
                        nc.tensor.matmul(
                            ps[:, half * NBH:(half + 1) * NBH],
                            w16[:, c * H:(c + 1) * H],
                            xt16[c][:, ff * NBH:(ff + 1) * NBH],
                            start=(c == 0), stop=(c == 1),
                        )
                nc.vector.tensor_copy(dst[:, f * NBH:(f + 2) * NBH], ps[:])

            def proj_qkT_head(w16, dst, f, on_act):
                # single half with its own copy (prologue: ACT is idle)
                ps = stp.tile([P, NB], f32, tag="stp", name=f"pjh_{f}")
                for c in range(2):
                    nc.tensor.matmul(
                        ps[:, :NBH],
                        w16[:, c * H:(c + 1) * H],
                        xt16[c][:, f * NBH:(f + 1) * NBH],
                        start=(c == 0), stop=(c == 1),
                    )
                if on_act:
                    nc.scalar.activation(
                        dst[:, f * NBH:(f + 1) * NBH], ps[:, :NBH], AF.Copy
                    )
                else:
                    nc.vector.tensor_copy(
                        dst[:, f * NBH:(f + 1) * NBH], ps[:, :NBH]
                    )

            def proj_w_batch(b):
                # w tiles 4b..4b+3 into one psum slot, one copy
                ps = stp.tile([P, NB], f32, tag="stp", name=f"pjw_{b}")
                for j4 in range(4):
                    j = 4 * b + j4
                    for c in range(2):
                        nc.tensor.matmul(
                            ps[:, j4 * H:(j4 + 1) * H],
                            xt16[c][:, j * P:(j + 1) * P],
                            vd16[:, c * H:(c + 1) * H],
                            start=(c == 0), stop=(c == 1),
                        )
                nc.vector.tensor_copy(
                    w_sb[:, b * NBH:(b + 1) * NBH], ps[:, :NBH]
                )

            def qk_exp(k, mt):
                ps = stp.tile([P, NB], f32, tag="stp", name=f"qk_{k}_{mt}")
                for h in range(2):
                    nc.tensor.matmul(
                        ps[:, h * NBH:(h + 1) * NBH],
                        kT16[:, mt * P:(mt + 1) * P],
                        qT16[:, k * NB + h * NBH: k * NB + (h + 1) * NBH],
                        start=True, stop=True,
                    )
                if mt % 2 == 0:
                    e = est_pool.tile([P, 2 * NB], bf16, tag="est",
                                      name=f"est_{k}_{mt // 2}")
                    est[(k, mt // 2)] = e
                else:
                    e = est[(k, mt // 2)]
                nc.scalar.activation(
                    e[:, (mt % 2) * NB:(mt % 2 + 1) * NB], ps[:],
                    AF.Exp, scale=1.0 / SCALER,
                )

            def tadd(k, name, a, b, dtype, width, bufs=None):
                tag = name.rstrip("0123456789") or name
                if bufs is None:
                    bufs = {"u": 3, "b": 2}.get(tag, 1)
                t = tree_pool.tile([P, width], dtype, tag=tag, bufs=bufs,
                                   name=f"{name}_{k}")
                nc.vector.tensor_add(t[:], a, b)
                tr[(k, name)] = t
                return t

            def tree_adds(k, mt):
                # 2048-wide pairwise tree over est pairs 0..14; pair 15
                # folds separately at the next block head (short tail)
                W2 = 2 * NB
                if mt % 4 == 3 and mt <= 27:
                    i = mt // 4
                    tadd(k, f"u{i}", est[(k, 2 * i)][:],
                         est[(k, 2 * i + 1)][:], bf16, W2)
                if mt == 7:
                    tadd(k, "b0", tr[(k, "u0")][:], tr[(k, "u1")][:], bf16, W2)
                if mt == 15:
                    tadd(k, "b1", tr[(k, "u2")][:], tr[(k, "u3")][:], bf16, W2)
                    tadd(k, "c0", tr[(k, "b0")][:], tr[(k, "b1")][:], bf16, W2)
                if mt == 29:
                    tadd(k, "b2", tr[(k, "u5")][:], tr[(k, "u6")][:], bf16, W2)
                    tadd(k, "d0", tr[(k, "u4")][:], est[(k, 14)][:], bf16, W2)
                    tadd(k, "e0", tr[(k, "b2")][:], tr[(k, "d0")][:], bf16, W2)
                if mt == 30:
                    t4 = tadd(k, "t4", tr[(k, "c0")][:], tr[(k, "e0")][:],
                              f32, W2)
                    t5p = tree_pool.tile([P, NB], f32, tag="t5p", bufs=2,
                                         name=f"t5p_{k}")
                    nc.vector.tensor_add(t5p[:], t4[:, 0:NB], t4[:, NB:2 * NB])
                    tr[(k, "t5p")] = t5p

            def fold_last(k):
                # fold the final pair (tiles 30,31) into the rowsum
                f15 = tree_pool.tile([P, NB], f32, tag="f15", bufs=1,
                                     name=f"f15_{k}")
                nc.vector.tensor_add(
                    f15[:], est[(k, 15)][:, 0:NB], est[(k, 15)][:, NB:2 * NB]
                )
                t = tree_pool.tile([P, NB], f32r, tag="t5", bufs=1,
                                   name=f"t5_{k}")
                nc.vector.tensor_add(t[:], tr[(k, "t5p")][:], f15[:])
                tr[(k, "t5")] = t

            def bc_chain(k):
                # partition-sum + broadcast in one all-ones fp32r matmul
                ps = stp.tile([P, NB], f32, tag="stp", name=f"bcm_{k}")
                for h in range(2):
                    nc.tensor.matmul(
                        ps[:, h * NBH:(h + 1) * NBH],
                        ones32[:],
                        tr[(k, "t5")][:, h * NBH:(h + 1) * NBH],
                        start=True, stop=True,
                    )
                bck = sb_small.tile([P, NB], f32, tag="bc", bufs=2,
                                    name=f"bc_{k}")
                nc.vector.reciprocal_approx_fast(bck[:], ps[:])
                bc[k] = bck

            def norm_mid(k):
                msc = sb_small.tile([P, NB], bf16, tag="msc", bufs=2,
                                    name=f"msc_{k}")
                nc.vector.tensor_copy(msc[:], mtiles[k][:])
                mscs[k] = msc

            def drain_out(k):
                # apply V_up, normalize by 1/rowsum, store transposed (f16)
                for lt in range(2):
                    op = stp.tile([P, NB], f32, tag="stp", name=f"op_{k}_{lt}")
                    for h in range(2):
                        nc.tensor.matmul(
                            op[:, h * NBH:(h + 1) * NBH],
                            vu_bf[:, lt * P:(lt + 1) * P],
                            mscs[k][:, h * NBH:(h + 1) * NBH],
                            start=True, stop=True,
                        )
                    fin = outfin_pool.tile([P, NB], f16, tag="fin")
                    nc.vector.tensor_mul(fin[:], op[:], bc[k][:])
                    nc.gpsimd.dma_start(
                        out_ext[lt * P:(lt + 1) * P, k * NB:(k + 1) * NB],
                        fin[:],
                    )

            def pv2(kk, j, mid):
                for h in range(2):
                    nc.tensor.matmul(
                        mid[:, h * NBH:(h + 1) * NBH],
                        w_sb[:, j * H:(j + 1) * H],
                        est_ap(kk, j, h),
                        start=(j == 0), stop=(j == MT - 1),
                    )

            # PE warm-up: junk matmuls while the input DMA is in flight
            for i in range(14):
                ps = stp.tile([P, NB], f32, tag="stp", name=f"warm_{i}")
                nc.tensor.matmul(
                    ps[:, :NBH], wrm[:, :P], wrm[:], start=True, stop=True
                )

            # head: the first QK tiles need qT/kT half-blocks 0,1 (s0)
            proj_qkT_head(qw16, qT16, 0, on_act=True)
            proj_qkT_head(qw16, qT16, 1, on_act=True)
            proj_qkT_head(kw16, kT16, 0, on_act=False)
            proj_qkT_head(kw16, kT16, 1, on_act=False)

            # Uniform half-block-lagged schedule: during block k the PE
            # runs QK(k) plus the oldest pending attention@w work; block 0
            # uses the batched projections as its filler.
            for k in range(NT):
                for mt in range(MT):
                    qk_exp(k, mt)
                    if k == 0:
                        if mt % 4 == 1 and mt <= 13:
                            proj_w_batch(mt // 4 * 2)
                            proj_w_batch(mt // 4 * 2 + 1)
                        if mt in (2, 10, 18):
                            proj_qkT_pair(kw16, kT16, mt // 8 * 2 + 2)
                        if mt == 15:
                            proj_qkT_pair(qw16, qT16, 2)
                        if mt == 19:
                            nc.vector.tensor_copy(vu_bf[:], vu16[:])
                    if k == 1 and mt in (2, 4):
                        proj_qkT_pair(qw16, qT16, mt + 2)
                    if k >= 1 and mt <= 15:
                        pv2(k - 1, 16 + mt, mtiles[k - 1])
                    if mt == 16:
                        mid = mtp.tile([P, NB], f32, tag="mtp",
                                       name=f"mid_{k}")
                        mtiles[k] = mid
                    if mt >= 16:
                        pv2(k, mt - 16, mtiles[k])
                    if k == NT - 1 and mt >= 24:
                        # last block: pull forward part of the epilogue
                        pv2(k, mt - 8, mtiles[k])
                    if k >= 1:
                        if mt == 0:
                            fold_last(k - 1)
                        if mt == 2:
                            bc_chain(k - 1)
                        if mt == 15:
                            norm_mid(k - 1)
                        if mt == 22:
                            drain_out(k - 1)
                    tree_adds(k, mt)

            # epilogue: finish block 3's product and drain it
            k3 = NT - 1
            for j in range(24, MT):
                pv2(k3, j, mtiles[k3])
            fold_last(k3)
            bc_chain(k3)
            norm_mid(k3)
            drain_out(k3)

    if not nc.is_finalized():
        nc.finalize()
    return nc


_GRAPH_CACHE = {}


def _get_graph():
    if "nc" not in _GRAPH_CACHE:
        _GRAPH_CACHE["nc"] = _build()
    return _GRAPH_CACHE["nc"]


def run(inputs: dict, trace: bool = False):
    """Run the SPMD kernel on 8 cores. Returns (output, BassKernelResults)."""
    from concourse.bass_utils import run_bass_kernel_spmd

    x = np.asarray(inputs["x"], dtype=np.float32)
    Q = np.asarray(inputs["Q"], dtype=np.float32)[0]
    K = np.asarray(inputs["K"], dtype=np.float32)[0]
    Vd = np.asarray(inputs["V_down"], dtype=np.float32)[0]
    Vu = np.asarray(inputs["V_up"], dtype=np.float32)[0]

    wq = np.ascontiguousarray(Q).astype(np.float16)
    wk = np.ascontiguousarray(K).astype(np.float16)
    vd = np.ascontiguousarray(Vd).astype(np.float16)
    vu = np.ascontiguousarray(Vu).astype(np.float16)

    in_maps = []
    for b in range(B):
        in_maps.append({
            "xT": np.ascontiguousarray(x[b].T).astype(np.float16),
            "Wq": wq,
            "Wk": wk,
            "Vd": vd,
            "Vu": vu,
        })

    nc = _get_graph()
    res = run_bass_kernel_spmd(nc, in_maps, core_ids=list(range(B)), trace=trace)
    # device output is [L, N] per core; un-transpose during the gather
    out = np.stack([np.asarray(res.results[i]["out"]).astype(np.float32).T for i in range(B)])
    return np.ascontiguousarray(out, dtype=np.float32), res


def kernel(**inputs) -> np.ndarray:
    out, _ = run(inputs, trace=False)
    return out


# revision 16
# speedup vs baseline: 1.8624x; 1.0401x over previous
"""Trainium2 Bass kernel for nn_Attention_27358941675773.

Reference computation (per batch b):
    q = x @ Q              [N, H]
    k = x @ K              [N, H]
    V = V_down @ V_up      [L, L]
    v = x @ V              [N, L]
    S = q @ k.T / 256      [N, N]
    out = softmax(S) @ v   [N, L]

Sharding: pure data-parallel over batch B=8 across the 8 NeuronCores
(one batch element per core); small params replicated. No collectives.

Per-core kernel design (N=4096, L=256, H=128):
  - Inputs shipped as fp16 (x transposed to [L, N]); all matmuls run at
    full PE rate. qT [H,N] and kT [H,N] are computed directly in
    transposed layout so scores are built as S_T[m, n] (keys on the
    partition axis) with no transposes anywhere in the pipeline.
  - Value path factored through the rank-H bottleneck:
        out = softmax(S) @ x @ V_down @ V_up
    so the O(N^2) product contracts into H=128 columns and V_up is
    applied after the softmax.
  - exp(S_T/256) runs on the Scalar engine straight out of PSUM in
    [128, 1024] tiles through a 3-slot PSUM ring (the scheduler's WAR
    semaphores release one exp late, so a ring depth of 3 is the
    minimum that keeps QK fully pipelined against the exp stream; wider
    exp tiles are geometrically impossible with 8 PSUM banks and the
    2-bank PV accumulator). The exp stream is the critical path.
  - exp output lands in PAIRED [128, 2048] est tiles (two exps fill one
    tile), letting the rowsum tree run 2048-wide bf16 adds on the
    Vector engine: 16 tree ops per block instead of 31, and the last
    pair folds separately so the post-last-exp chain is short.
  - Partition-axis rowsum reduce+broadcast in ONE matmul with an
    all-ones [128,128] fp32r stationary operand (full PE rate),
    replacing the 8.3us GpSimd PartitionAllReduce.
  - The numerator copy (mid -> SBUF) and normalization ride the Vector
    engine; the Scalar engine runs nothing but the exp stream.
  - DMA: weights first as single strided descriptors on the GpSimd
    queue, then x in 4 large descriptors on the SP queue, so the
    projection weights never queue behind the 2MB x transfer.
  - Uniform half-block-lagged schedule: per key tile the PE runs 2 QK
    matmuls plus 2 lagged attention@w matmuls; block 0 uses the
    projections (batched 4-8 to a PSUM slot) as its filler; junk
    matmuls warm the PE clock gate during the input DMA.
  - Output stored transposed [L, N] fp16; host un-transposes on gather.
"""

import os
import sys

import numpy as np

for _p in ("/opt/trn_rl_repo",):
    if _p not in sys.path and os.path.isdir(_p):
        sys.path.insert(0, _p)

B, N, L, H = 8, 4096, 256, 128
SCALER = 256.0
NB = 1024           # query-block (free dim of score tiles)
NBH = 512           # half block (one PSUM bank of fp32)
NT = N // NB        # 4 query blocks
MT = N // 128       # 32 key tiles of 128
P = 128


def _build():
    import concourse.bass as bass
    import concourse.tile as tile
    from concourse import bacc, bass_isa, mybir
    from contextlib import ExitStack

    f32 = mybir.dt.float32
    f32r = mybir.dt.float32r
    f16 = mybir.dt.float16
    bf16 = mybir.dt.bfloat16
    AF = mybir.ActivationFunctionType

    nc = bacc.Bacc(
        "TRN2", target_bir_lowering=False, debug=False, num_devices=B
    )

    xT_ext = nc.declare_dram_parameter("xT", [L, N], f16, isOutput=False)
    wq_ext = nc.declare_dram_parameter("Wq", [L, H], f16, isOutput=False)
    wk_ext = nc.declare_dram_parameter("Wk", [L, H], f16, isOutput=False)
    vd_ext = nc.declare_dram_parameter("Vd", [L, H], f16, isOutput=False)
    vu_ext = nc.declare_dram_parameter("Vu", [H, L], f16, isOutput=False)
    # output stored transposed [L, N]; host un-transposes at gather
    out_ext = nc.declare_dram_parameter("out", [L, N], f16, isOutput=True)

    with tile.TileContext(nc) as tc, ExitStack() as ctx:
        persist = ctx.enter_context(tc.tile_pool(name="persist", bufs=1))

        ones32f = persist.tile([P, P], f32)
        nc.gpsimd.memset(ones32f[:], 1.0)
        ones32 = persist.tile([P, P], f32r)
        nc.vector.tensor_copy(ones32[:], ones32f[:])
        # touch Exp right away so the ~2.7us ACT table load overlaps the
        # input DMAs instead of delaying the first real exp
        dum = persist.tile([1, 2], f32)
        nc.gpsimd.memset(dum[:], 0.0)
        nc.scalar.activation(dum[:, 1:2], dum[:, 0:1], AF.Exp)
        wrm = persist.tile([P, NBH], bf16, name="wrm")
        nc.vector.memset(wrm[:], 0.0)

        qw16 = persist.tile([P, 2 * H], f16)    # Q   [l_chunk][l_in, h]
        kw16 = persist.tile([P, 2 * H], f16)
        vd16 = persist.tile([P, 2 * H], f16)    # V_down [l_chunk][l_in, h]
        vu16 = persist.tile([P, L], f16)        # V_up   [h, l]
        vu_bf = persist.tile([P, L], bf16)      # V_up as bf16 (out matmul)
        xt16 = [persist.tile([P, N], f16, name=f"xt16_{c}") for c in range(2)]
        qT16 = persist.tile([P, N], f16)        # q.T       [h, n]
        kT16 = persist.tile([P, N], f16)        # k.T       [h, m]
        w_sb = persist.tile([P, MT * H], bf16)  # x@V_down  [m_tile][m_in, h]

        # ---------------- phase A: direct fp16 loads ----------------
        # weights first (single strided descriptors on the GpSimd queue)
        # so they never queue behind the 2MB x transfer; x s0 chunks next
        # (critical path for the first QK tiles), then the rest of x.
        def dma_w(w_ext, w_sbuf):
            nc.gpsimd.dma_start(
                w_sbuf[:].rearrange("p (c h) -> p c h", c=2),
                w_ext[:, :].rearrange("(c p) h -> p c h", c=2),
            )
        dma_w(wq_ext, qw16)
        for c in range(2):
            nc.sync.dma_start(
                xt16[c][:, 0:NB], xT_ext[c * P:(c + 1) * P, 0:NB]
            )
        dma_w(wk_ext, kw16)
        dma_w(vd_ext, vd16)
        for c in range(2):
            nc.sync.dma_start(
                xt16[c][:, NB:N], xT_ext[c * P:(c + 1) * P, NB:N]
            )
        nc.gpsimd.dma_start(vu16[:], vu_ext[:, :])

        # ------------- phases B+C: projections fused with attention -------
        with (
            tc.tile_pool(name="est", bufs=20) as est_pool,
            tc.tile_pool(name="tree", bufs=2) as tree_pool,
            tc.tile_pool(name="sb_small", bufs=2) as sb_small,
            tc.tile_pool(name="outfin", bufs=4) as outfin_pool,
            tc.tile_pool(name="stp", bufs=3, space="PSUM") as stp,
            tc.tile_pool(name="mtp", bufs=1, space="PSUM") as mtp,
        ):
            est = {}      # (k, pair) -> bf16 [128, 2048] exp tiles (2 halves)
            mtiles = {}   # k -> psum numerator mid^T [h, n] tile
            mscs = {}     # k -> normalized mid (bf16, SBUF)
            bc = {}       # k -> [128, NB] f32 broadcast 1/rowsum
            tr = {}       # tree tiles by (k, name)

            def est_ap(k, j, h):
                off = (j % 2) * NB + h * NBH
                return est[(k, j // 2)][:, off:off + NBH]

            def proj_qkT_pair(w16, dst, f):
                # projects halves f and f+1 into one psum slot, one copy
                ps = stp.tile([P, NB], f32, tag="stp", name=f"pjp_{f}")
                for half in range(2):
                    ff = f + half
                    for c in range(2):
                        nc.tensor.matmul(
                            ps[:, half * NBH:(half + 1) * NBH],
                            w16[:, c * H:(c + 1) * H],
                            xt16[c][:, ff * NBH:(ff + 1) * NBH],
                            start=(c == 0), stop=(c == 1),
                        )
                nc.vector.tensor_copy(dst[:, f * NBH:(f + 2) * NBH], ps[:])

            def proj_qkT_head(w16, dst, f, on_act):
                # single half with its own copy (prologue: ACT is idle)
                ps = stp.tile([P, NB], f32, tag="stp", name=f"pjh_{f}")
                for c in range(2):
                    nc.tensor.matmul(
                        ps[:, :NBH],
                        w16[:, c * H:(c + 1) * H],
                        xt16[c][:, f * NBH:(f + 1) * NBH],
                        start=(c == 0), stop=(c == 1),
                    )
                if on_act:
                    nc.scalar.activation(
                        dst[:, f * NBH:(f + 1) * NBH], ps[:, :NBH], AF.Copy
                    )
                else:
                    nc.vector.tensor_copy(
                        dst[:, f * NBH:(f + 1) * NBH], ps[:, :NBH]
                    )

            def proj_w_batch(b):
                # w tiles 4b..4b+3 into one psum slot, one copy
                ps = stp.tile([P, NB], f32, tag="stp", name=f"pjw_{b}")
                for j4 in range(4):
                    j = 4 * b + j4
                    for c in range(2):
                        nc.tensor.matmul(
                            ps[:, j4 * H:(j4 + 1) * H],
                            xt16[c][:, j * P:(j + 1) * P],
                            vd16[:, c * H:(c + 1) * H],
                            start=(c == 0), stop=(c == 1),
                        )
                nc.vector.tensor_copy(
                    w_sb[:, b * NBH:(b + 1) * NBH], ps[:, :NBH]
                )

            def qk_exp(k, mt):
                ps = stp.tile([P, NB], f32, tag="stp", name=f"qk_{k}_{mt}")
                for h in range(2):
                    nc.tensor.matmul(
                        ps[:, h * NBH:(h + 1) * NBH],
                        kT16[:, mt * P:(mt + 1) * P],
                        qT16[:, k * NB + h * NBH: k * NB + (h + 1) * NBH],
                        start=True, stop=True,
                    )
                if mt % 2 == 0:
                    e = est_pool.tile([P, 2 * NB], bf16, tag="est",
                                      name=f"est_{k}_{mt // 2}")
                    est[(k, mt // 2)] = e
                else:
                    e = est[(k, mt // 2)]
                nc.scalar.activation(
                    e[:, (mt % 2) * NB:(mt % 2 + 1) * NB], ps[:],
                    AF.Exp, scale=1.0 / SCALER,
                )

            def tadd(k, name, a, b, dtype, width, bufs=None):
                tag = name.rstrip("0123456789") or name
                if bufs is None:
                    bufs = {"u": 3, "b": 2}.get(tag, 1)
                t = tree_pool.tile([P, width], dtype, tag=tag, bufs=bufs,
                                   name=f"{name}_{k}")
                nc.vector.tensor_add(t[:], a, b)
                tr[(k, name)] = t
                return t

            def tree_adds(k, mt):
                # 2048-wide pairwise tree over est pairs 0..14; pair 15
                # folds separately at the next block head (short tail)
                W2 = 2 * NB
                if mt % 4 == 3 and mt <= 27:
                    i = mt // 4
                    tadd(k, f"u{i}", est[(k, 2 * i)][:],
                         est[(k, 2 * i + 1)][:], bf16, W2)
                if mt == 7:
                    tadd(k, "b0", tr[(k, "u0")][:], tr[(k, "u1")][:], bf16, W2)
                if mt == 15:
                    tadd(k, "b1", tr[(k, "u2")][:], tr[(k, "u3")][:], bf16, W2)
                    tadd(k, "c0", tr[(k, "b0")][:], tr[(k, "b1")][:], bf16, W2)
                if mt == 23:
                    tadd(k, "d0", tr[(k, "u4")][:], tr[(k, "u5")][:], bf16, W2)
                if mt == 27:
                    tadd(k, "e0", tr[(k, "d0")][:], tr[(k, "u6")][:], bf16, W2)
                if mt == 29:
                    tadd(k, "g0", tr[(k, "e0")][:], est[(k, 14)][:], bf16, W2)
                    tadd(k, "t4", tr[(k, "c0")][:], tr[(k, "g0")][:], bf16, W2)
                if mt == 30:
                    t4 = tr[(k, "t4")]
                    t5p = tree_pool.tile([P, NB], f32, tag="t5p", bufs=2,
                                         name=f"t5p_{k}")
                    nc.vector.tensor_add(t5p[:], t4[:, 0:NB], t4[:, NB:2 * NB])
                    tr[(k, "t5p")] = t5p

            def fold_last(k):
                # fold the final pair (tiles 30,31) into the rowsum
                f15 = tree_pool.tile([P, NB], f32, tag="f15", bufs=1,
                                     name=f"f15_{k}")
                nc.vector.tensor_add(
                    f15[:], est[(k, 15)][:, 0:NB], est[(k, 15)][:, NB:2 * NB]
                )
                t = tree_pool.tile([P, NB], f32r, tag="t5", bufs=1,
                                   name=f"t5_{k}")
                nc.vector.tensor_add(t[:], tr[(k, "t5p")][:], f15[:])
                tr[(k, "t5")] = t

            def bc_chain(k):
                # partition-sum + broadcast in one all-ones fp32r matmul
                ps = stp.tile([P, NB], f32, tag="stp", name=f"bcm_{k}")
                for h in range(2):
                    nc.tensor.matmul(
                        ps[:, h * NBH:(h + 1) * NBH],
                        ones32[:],
                        tr[(k, "t5")][:, h * NBH:(h + 1) * NBH],
                        start=True, stop=True,
                    )
                bck = sb_small.tile([P, NB], f32, tag="bc", bufs=2,
                                    name=f"bc_{k}")
                nc.vector.reciprocal_approx_fast(bck[:], ps[:])
                bc[k] = bck

            def norm_mid(k):
                msc = sb_small.tile([P, NB], bf16, tag="msc", bufs=2,
                                    name=f"msc_{k}")
                nc.vector.tensor_copy(msc[:], mtiles[k][:])
                mscs[k] = msc

            def drain_out(k):
                # apply V_up, normalize by 1/rowsum, store transposed (f16)
                for lt in range(2):
                    op = stp.tile([P, NB], f32, tag="stp", name=f"op_{k}_{lt}")
                    for h in range(2):
                        nc.tensor.matmul(
                            op[:, h * NBH:(h + 1) * NBH],
                            vu_bf[:, lt * P:(lt + 1) * P],
                            mscs[k][:, h * NBH:(h + 1) * NBH],
                            start=True, stop=True,
                        )
                    fin = outfin_pool.tile([P, NB], f16, tag="fin")
                    nc.vector.tensor_mul(fin[:], op[:], bc[k][:])
                    nc.gpsimd.dma_start(
                        out_ext[lt * P:(lt + 1) * P, k * NB:(k + 1) * NB],
                        fin[:],
                    )

            def pv2(kk, j, mid):
                for h in range(2):
                    nc.tensor.matmul(
                        mid[:, h * NBH:(h + 1) * NBH],
                        w_sb[:, j * H:(j + 1) * H],
                        est_ap(kk, j, h),
                        start=(j == 0), stop=(j == MT - 1),
                    )

            # PE warm-up: junk matmuls while the input DMA is in flight
            for i in range(14):
                ps = stp.tile([P, NB], f32, tag="stp", name=f"warm_{i}")
                nc.tensor.matmul(
                    ps[:, :NBH], wrm[:, :P], wrm[:], start=True, stop=True
                )

            # head: the first QK tiles need qT/kT half-blocks 0,1 (s0)
            proj_qkT_head(qw16, qT16, 0, on_act=True)
            proj_qkT_head(qw16, qT16, 1, on_act=True)
            proj_qkT_head(kw16, kT16, 0, on_act=False)
            proj_qkT_head(kw16, kT16, 1, on_act=False)

            # Uniform half-block-lagged schedule: during block k the PE
            # runs QK(k) plus the oldest pending attention@w work; block 0
            # uses the batched projections as its filler.
            for k in range(NT):
                for mt in range(MT):
                    qk_exp(k, mt)
                    if k == 0:
                        if mt % 4 == 1 and mt <= 13:
                            proj_w_batch(mt // 4 * 2)
                            proj_w_batch(mt // 4 * 2 + 1)
                        if mt in (2, 10, 18):
                            proj_qkT_pair(kw16, kT16, mt // 8 * 2 + 2)
                        if mt == 15:
                            proj_qkT_pair(qw16, qT16, 2)
                        if mt == 19:
                            nc.gpsimd.tensor_copy(vu_bf[:], vu16[:])
                    if k == 1 and mt in (8, 12):
                        proj_qkT_pair(qw16, qT16, (mt - 8) // 2 + 4)
                    if k >= 1 and mt <= 15:
                        pv2(k - 1, 16 + mt, mtiles[k - 1])
                    if mt == 16:
                        mid = mtp.tile([P, NB], f32, tag="mtp",
                                       name=f"mid_{k}")
                        mtiles[k] = mid
                    if mt >= 16:
                        pv2(k, mt - 16, mtiles[k])
                    if k == NT - 1 and mt >= 20:
                        # last block: pull forward part of the epilogue
                        pv2(k, mt - 4, mtiles[k])
                    if k >= 1:
                        if mt == 0:
                            fold_last(k - 1)
                        if mt == 2:
                            bc_chain(k - 1)
                        if mt == 15:
                            norm_mid(k - 1)
                        if mt == 22:
                            drain_out(k - 1)
                    tree_adds(k, mt)

            # epilogue: finish block 3's product and drain it
            k3 = NT - 1
            for j in range(28, MT):
                pv2(k3, j, mtiles[k3])
            fold_last(k3)
            bc_chain(k3)
            norm_mid(k3)
            drain_out(k3)

    if not nc.is_finalized():
        nc.finalize()
    return nc


_GRAPH_CACHE = {}


def _get_graph():
    if "nc" not in _GRAPH_CACHE:
        _GRAPH_CACHE["nc"] = _build()
    return _GRAPH_CACHE["nc"]


def run(inputs: dict, trace: bool = False):
    """Run the SPMD kernel on 8 cores. Returns (output, BassKernelResults)."""
    from concourse.bass_utils import run_bass_kernel_spmd

    x = np.asarray(inputs["x"], dtype=np.float32)
    Q = np.asarray(inputs["Q"], dtype=np.float32)[0]
    K = np.asarray(inputs["K"], dtype=np.float32)[0]
    Vd = np.asarray(inputs["V_down"], dtype=np.float32)[0]
    Vu = np.asarray(inputs["V_up"], dtype=np.float32)[0]

    wq = np.ascontiguousarray(Q).astype(np.float16)
    wk = np.ascontiguousarray(K).astype(np.float16)
    vd = np.ascontiguousarray(Vd).astype(np.float16)
    vu = np.ascontiguousarray(Vu).astype(np.float16)

    in_maps = []
    for b in range(B):
        in_maps.append({
            "xT": np.ascontiguousarray(x[b].T).astype(np.float16),
            "Wq": wq,
            "Wk": wk,
            "Vd": vd,
            "Vu": vu,
        })

    nc = _get_graph()
    res = run_bass_kernel_spmd(nc, in_maps, core_ids=list(range(B)), trace=trace)
    # device output is [L, N] per core; un-transpose during the gather
    out = np.stack([np.asarray(res.results[i]["out"]).astype(np.float32).T for i in range(B)])
    return np.ascontiguousarray(out, dtype=np.float32), res


def kernel(**inputs) -> np.ndarray:
    out, _ = run(inputs, trace=False)
    return out
